# revision 19
# baseline (speedup 1.0000x reference)
"""MoE block (E=8, top-2, D=1024, P=4096, T=4096) on 8 TRN2 NeuronCores.

Strategy: expert-parallel. The router (0.03% of FLOPs) runs on host to
produce the token->expert dispatch; core e receives the tokens routed to
expert e (gathered, transposed, bf16), runs the expert MLP
  y = (gelu_tanh(x @ W1 + b1) @ W2 + b2) * router_weight
entirely on device, and the host scatter-adds the per-expert outputs back
into the full [T, D] output.

Device kernel (per core, SPMD):
  phase 1: H[p, t] = gelu(sum_d W1[d, p] xT[d, t] + b1[p])   (H kept in SBUF, bf16)
  phase 2: y[t, d] = (sum_p H[p, t] W2[p, d] + b2[d]) * wt[t]
b2 is added via a K=1 ones-row matmul into the same PSUM accumulation
group; the router weight is applied as a per-partition activation scale.

All DRAM inputs are pre-swizzled on host so every device DMA reads fully
contiguous per-partition runs (the partition index p is the SLOWEST axis,
matching SBUF tile layout):
  xT_d [128, DK*C]     xT_d[p, chunk-major (dk, c)] = x_g[c, dk*128+p]
  W1_d [128, DK*P]     blocks of [DK, 2*128] per pk-pair (pkg-major)
  W2_d [128, PK*D]     W2_d[p, pk*D + d] = W2[pk*128+p, d]
  b1_d [128, PK]       b1_d[p, pk] = b1[pk*128+p]
  wt_d [128, TT]       wt_d[p, tt] = w[tt*128+p]
"""

import numpy as np
import ml_dtypes

E = 8
K = 2
D = 1024
P = 4096
NCORES = 8

DK = D // 128   # 8
PK = P // 128   # 32

BF16 = ml_dtypes.bfloat16

_NC_CACHE = {}


def _route(xf, Wr, br):
    """Top-2 routing + softmax weights, matching the jax reference."""
    scores = xf @ Wr + br                                   # [T, E] fp32
    idx = np.argsort(-scores, axis=-1, kind="stable")[:, :K]  # [T, K]
    top = np.take_along_axis(scores, idx, axis=-1)          # [T, K]
    m = top.max(axis=-1, keepdims=True)
    ex = np.exp(top - m)
    w = ex / ex.sum(axis=-1, keepdims=True)                 # [T, K]
    return idx, w


def _token_chunks(C):
    """Split C into free-dim chunks of <=512 for fp32 PSUM banks.

    The first chunk is only 128 tokens so the very first matmul group
    depends on a minimal amount of DMA'd data.
    """
    chunks = [(0, 128)]
    c0 = 128
    while c0 < C:
        cn = min(512, C - c0)
        chunks.append((c0, cn))
        c0 += cn
    return chunks


def _build_nc(C, act_fn=None):
    """Build the per-core Bass graph for capacity-C tokens."""
    import concourse.bass as bass  # noqa: F401
    import concourse.mybir as mybir
    import concourse.tile as tile
    from concourse import bacc

    dt = mybir.dt
    AF = mybir.ActivationFunctionType
    if act_fn is None:
        act_fn = AF.Gelu_apprx_tanh

    TT = C // 128    # token tiles in phase 2
    ND = D // 512    # 2 output d-chunks in phase 2
    PKG = PK // 2    # W1 streamed in pk-pairs for 4KB-contiguous DMA

    nc = bacc.Bacc(None, target_bir_lowering=False)

    xT = nc.dram_tensor("xT", [128, DK * C], dt.bfloat16, kind="ExternalInput")
    W1 = nc.dram_tensor("W1", [128, DK * P], dt.bfloat16, kind="ExternalInput")
    b1 = nc.dram_tensor("b1", [128, PK], dt.float32, kind="ExternalInput")
    W2 = nc.dram_tensor("W2", [128, PK * D], dt.bfloat16, kind="ExternalInput")
    wt = nc.dram_tensor("wt", [128, TT], dt.float32, kind="ExternalInput")
    y = nc.dram_tensor("y", [C, D], dt.float32, kind="ExternalOutput")

    chunks = _token_chunks(C)

    with tile.TileContext(nc) as tc:
        with (
            tc.tile_pool(name="xpool", bufs=1) as xpool,
            tc.tile_pool(name="w1pool", bufs=4) as w1pool,
            tc.tile_pool(name="w2pool", bufs=1) as w2pool,
            tc.tile_pool(name="hpool", bufs=1) as hpool,
            tc.tile_pool(name="cpool", bufs=1) as cpool,
            tc.tile_pool(name="ypool", bufs=3) as ypool,
            tc.tile_pool(name="psum", bufs=8, space="PSUM") as psum_pool,
        ):
            H_sb = hpool.tile([128, PK, C], dt.bfloat16)
            W2_sb = w2pool.tile([128, PK, D], dt.bfloat16)

            # PE warm-up: ~5us of dummy matmuls with no DMA dependency so
            # the HAM clock-gate opens (1.2 -> 2.4 GHz) while the first
            # real tiles are still in flight on the DMA rings.
            warm_sb = cpool.tile([128, 512], dt.bfloat16)
            nc.any.memset(warm_sb[:], 0.0)
            ps_w = psum_pool.tile(
                [128, 512], dt.float32, tag="ps", name="ps_warm"
            )
            NWARM = 20
            for i in range(NWARM):
                nc.tensor.matmul(
                    ps_w[:],
                    lhsT=warm_sb[:, :128],
                    rhs=warm_sb[:],
                    start=(i == 0),
                    stop=(i == NWARM - 1),
                )

            # First W1 pair at the very head of the sync ring so matmuls
            # can start as early as possible.
            w1_tiles = {}
            w1_tiles[0] = w1pool.tile(
                [128, DK, 256], dt.bfloat16, tag="w1", name="w1_t0"
            )
            nc.sync.dma_start(w1_tiles[0][:], W1[:, 0 : DK * 256])

            # per-chunk xT tiles: both DMA sides fully contiguous (8KB runs)
            xT_tiles = []
            for i, (c0, cn) in enumerate(chunks):
                xc = xpool.tile(
                    [128, DK, cn], dt.bfloat16, tag=f"xc{i}", name=f"xc{i}"
                )
                nc.sync.dma_start(
                    xc[:],
                    xT[:, DK * c0 : DK * (c0 + cn)].rearrange(
                        "p (dk c) -> p dk c", dk=DK
                    ),
                )
                xT_tiles.append(xc)
            b1_sb = cpool.tile([128, PK], dt.float32)
            nc.sync.dma_start(b1_sb[:], b1[:])
            wt_sb = cpool.tile([128, TT], dt.float32)
            nc.sync.dma_start(wt_sb[:], wt[:])

            # ---- Phase 1: H = gelu(W1.T x + b1), H layout [p_dim, tokens]
            for pkg in range(PKG):
                if pkg not in w1_tiles:
                    w1_tiles[pkg] = w1pool.tile(
                        [128, DK, 256], dt.bfloat16, tag="w1", name=f"w1_t{pkg}"
                    )
                    nc.sync.dma_start(
                        w1_tiles[pkg][:],
                        W1[:, pkg * DK * 256 : (pkg + 1) * DK * 256],
                    )
                w1_sb = w1_tiles[pkg]
                for j in range(2):
                    pk = 2 * pkg + j
                    for ci, (c0, cn) in enumerate(chunks):
                        ps = psum_pool.tile([128, 512], dt.float32)
                        for dk in range(DK):
                            nc.tensor.matmul(
                                ps[:, :cn],
                                lhsT=w1_sb[:, dk, j * 128 : (j + 1) * 128],
                                rhs=xT_tiles[ci][:, dk, :],
                                start=(dk == 0),
                                stop=(dk == DK - 1),
                            )
                        nc.scalar.activation(
                            H_sb[:, pk, c0 : c0 + cn],
                            ps[:, :cn],
                            act_fn,
                            bias=b1_sb[:, pk : pk + 1],
                        )
                # W2: two pk-chunks per pkg iteration on the scalar ring,
                # emitted after this pkg's activations so the early ring
                # bandwidth goes to W1/xT.
                nc.scalar.dma_start(
                    W2_sb[:, 2 * pkg : 2 * pkg + 2, :],
                    W2[:, 2 * pkg * D : (2 * pkg + 2) * D].rearrange(
                        "p (k d) -> p k d", k=2
                    ),
                )

            # ---- Phase 2: y = (H.T W2) * wt, tokens on partitions
            # (b2 is folded into the host-side combine)
            for tt in range(TT):
                y_sb = ypool.tile([128, D], dt.float32)
                for dn in range(ND):
                    ps = psum_pool.tile([128, 512], dt.float32)
                    for pk in range(PK):
                        nc.tensor.matmul(
                            ps[:],
                            lhsT=H_sb[:, pk, tt * 128 : (tt + 1) * 128],
                            rhs=W2_sb[:, pk, dn * 512 : (dn + 1) * 512],
                            start=(pk == 0),
                            stop=(pk == PK - 1),
                        )
                    nc.scalar.activation(
                        y_sb[:, dn * 512 : (dn + 1) * 512],
                        ps[:],
                        AF.Copy,
                        scale=wt_sb[:, tt : tt + 1],
                    )
                    nc.sync.dma_start(
                        y[tt * 128 : (tt + 1) * 128, dn * 512 : (dn + 1) * 512],
                        y_sb[:, dn * 512 : (dn + 1) * 512],
                    )

    nc.finalize()
    return nc


def _get_nc(C):
    if C not in _NC_CACHE:
        _NC_CACHE[C] = _build_nc(C)
    return _NC_CACHE[C]


def _pack_inputs(xf, te, we, W1e, b1e, W2e, C):
    """Host-side swizzle of one expert's shard into device DRAM layouts."""
    n_e = len(te)
    chunks = _token_chunks(C)

    # xT: [128, DK*C], chunk-major [dk, c] blocks
    xg = np.zeros((C, D), dtype=np.float32)
    xg[:n_e] = xf[te]
    xt = xg.T.astype(BF16).reshape(DK, 128, C)        # [dk, p, c]
    xT_d = np.empty((128, DK * C), dtype=BF16)
    col = 0
    for c0, cn in chunks:
        blk = xt[:, :, c0 : c0 + cn]                  # [dk, p, cn]
        xT_d[:, col : col + DK * cn] = (
            blk.transpose(1, 0, 2).reshape(128, DK * cn)
        )
        col += DK * cn

    # W1: [128, DK*P] as pkg-major blocks [dk, 256]
    w1 = W1e.astype(BF16).reshape(DK, 128, PK // 2, 256)  # [dk, p, pkg, m]
    W1_d = np.ascontiguousarray(
        w1.transpose(1, 2, 0, 3).reshape(128, (PK // 2) * DK * 256)
    )

    # W2: [128, PK*D]
    w2 = W2e.astype(BF16).reshape(PK, 128, D)             # [pk, p, d]
    W2_d = np.ascontiguousarray(w2.transpose(1, 0, 2).reshape(128, PK * D))

    b1_d = np.ascontiguousarray(b1e.astype(np.float32).reshape(PK, 128).T)

    wt_full = np.zeros((C,), dtype=np.float32)
    wt_full[:n_e] = we
    wt_d = np.ascontiguousarray(wt_full.reshape(C // 128, 128).T)

    return {
        "xT": xT_d,
        "W1": W1_d,
        "b1": b1_d,
        "W2": W2_d,
        "wt": wt_d,
    }


def kernel(x, W1, b1, W2, b2, Wr, br):
    from concourse.bass_utils import run_bass_kernel_spmd

    x = np.asarray(x)
    B, S, _ = x.shape
    T = B * S
    xf = np.ascontiguousarray(x.reshape(T, D).astype(np.float32))

    idx, w = _route(xf, np.asarray(Wr, np.float32), np.asarray(br, np.float32))

    # Per-expert token lists
    sel = []
    for e in range(E):
        mask = (idx == e).any(axis=1)
        te = np.nonzero(mask)[0]
        ke = (idx[te] == e).argmax(axis=1)
        we = w[te, ke]
        sel.append((te, we))

    maxn = max(len(te) for te, _ in sel)
    C = ((maxn + 127) // 128) * 128

    nc = _get_nc(C)

    W1f = np.asarray(W1)
    W2f = np.asarray(W2)
    b1f = np.asarray(b1, np.float32)
    b2f = np.asarray(b2, np.float32)

    in_maps = []
    for e in range(E):
        te, we = sel[e]
        in_maps.append(_pack_inputs(xf, te, we, W1f[e], b1f[e], W2f[e], C))

    res = run_bass_kernel_spmd(nc, in_maps, core_ids=list(range(NCORES)))
    global LAST_RESULT
    LAST_RESULT = res

    # Combine: sum per-expert wt*(H@W2) shards, then add the router-weighted
    # b2 term (sum_e w[t,e]*b2[e]) in one tiny [T,E]@[E,D] matmul.
    out = np.zeros((T, D), dtype=np.float32)
    for e in range(E):
        te, _ = sel[e]
        out[te] += res.results[e]["y"][: len(te)]
    w_full = np.zeros((T, E), dtype=np.float32)
    np.put_along_axis(w_full, idx, w, axis=1)
    out += w_full @ b2f
    return out.reshape(B, S, D)


# revision 23
# speedup vs baseline: 1.0301x; 1.0301x over previous
"""MoE block (E=8, top-2, D=1024, P=4096, T=4096) on 8 TRN2 NeuronCores.

Strategy: expert-parallel. The router (0.03% of FLOPs) runs on host to
produce the token->expert dispatch; core e receives the tokens routed to
expert e (gathered, transposed, bf16), runs the expert MLP
  y = (gelu_tanh(x @ W1 + b1) @ W2 + b2) * router_weight
entirely on device, and the host scatter-adds the per-expert outputs back
into the full [T, D] output.

Device kernel (per core, SPMD):
  phase 1: H[p, t] = gelu(sum_d W1[d, p] xT[d, t] + b1[p])   (H kept in SBUF, bf16)
  phase 2: y[t, d] = (sum_p H[p, t] W2[p, d] + b2[d]) * wt[t]
b2 is added via a K=1 ones-row matmul into the same PSUM accumulation
group; the router weight is applied as a per-partition activation scale.

All DRAM inputs are pre-swizzled on host so every device DMA reads fully
contiguous per-partition runs (the partition index p is the SLOWEST axis,
matching SBUF tile layout):
  xT_d [128, DK*C]     xT_d[p, chunk-major (dk, c)] = x_g[c, dk*128+p]
  W1_d [128, DK*P]     blocks of [DK, 2*128] per pk-pair (pkg-major)
  W2_d [128, PK*D]     W2_d[p, pk*D + d] = W2[pk*128+p, d]
  b1_d [128, PK]       b1_d[p, pk] = b1[pk*128+p]
  wt_d [128, TT]       wt_d[p, tt] = w[tt*128+p]
"""

import numpy as np
import ml_dtypes

E = 8
K = 2
D = 1024
P = 4096
NCORES = 8

DK = D // 128   # 8
PK = P // 128   # 32

BF16 = ml_dtypes.bfloat16

_NC_CACHE = {}


def _route(xf, Wr, br):
    """Top-2 routing + softmax weights, matching the jax reference."""
    scores = xf @ Wr + br                                   # [T, E] fp32
    idx = np.argsort(-scores, axis=-1, kind="stable")[:, :K]  # [T, K]
    top = np.take_along_axis(scores, idx, axis=-1)          # [T, K]
    m = top.max(axis=-1, keepdims=True)
    ex = np.exp(top - m)
    w = ex / ex.sum(axis=-1, keepdims=True)                 # [T, K]
    return idx, w


def _token_chunks(C):
    """Split C into free-dim chunks of <=512 for fp32 PSUM banks.

    The first chunk is only 128 tokens so the very first matmul group
    depends on a minimal amount of DMA'd data.
    """
    chunks = [(0, 128)]
    c0 = 128
    while c0 < C:
        cn = min(512, C - c0)
        chunks.append((c0, cn))
        c0 += cn
    return chunks


def _build_nc(C, act_fn=None):
    """Build the per-core Bass graph for capacity-C tokens."""
    import concourse.bass as bass  # noqa: F401
    import concourse.mybir as mybir
    import concourse.tile as tile
    from concourse.tile import add_dep_helper
    from concourse import bacc

    dt = mybir.dt
    AF = mybir.ActivationFunctionType
    if act_fn is None:
        act_fn = AF.Gelu_apprx_tanh

    TT = C // 128    # token tiles in phase 2
    ND = D // 512    # 2 output d-chunks in phase 2
    PKG = PK // 2    # W1 streamed in pk-pairs for 4KB-contiguous DMA

    nc = bacc.Bacc(None, target_bir_lowering=False)

    xT = nc.dram_tensor("xT", [128, DK * C], dt.bfloat16, kind="ExternalInput")
    W1 = nc.dram_tensor("W1", [128, DK * P], dt.bfloat16, kind="ExternalInput")
    b1 = nc.dram_tensor("b1", [128, PK], dt.float32, kind="ExternalInput")
    W2 = nc.dram_tensor("W2", [128, PK * D], dt.bfloat16, kind="ExternalInput")
    wt = nc.dram_tensor("wt", [128, TT], dt.float32, kind="ExternalInput")
    y = nc.dram_tensor("y", [C, D], dt.float32, kind="ExternalOutput")

    chunks = _token_chunks(C)

    with tile.TileContext(nc) as tc:
        with (
            tc.tile_pool(name="xpool", bufs=1) as xpool,
            tc.tile_pool(name="w1pool", bufs=4) as w1pool,
            tc.tile_pool(name="w2pool", bufs=1) as w2pool,
            tc.tile_pool(name="hpool", bufs=1) as hpool,
            tc.tile_pool(name="cpool", bufs=1) as cpool,
            tc.tile_pool(name="ypool", bufs=3) as ypool,
            tc.tile_pool(name="psum", bufs=8, space="PSUM") as psum_pool,
        ):
            H_sb = hpool.tile([128, PK, C], dt.bfloat16)
            W2_sb = w2pool.tile([128, PK, D], dt.bfloat16)

            # PE warm-up: ~5us of dummy matmuls with no DMA dependency so
            # the HAM clock-gate opens (1.2 -> 2.4 GHz) while the first
            # real tiles are still in flight on the DMA rings.
            warm_sb = cpool.tile([128, 512], dt.bfloat16)
            nc.any.memset(warm_sb[:], 0.0)
            ps_w = psum_pool.tile(
                [128, 512], dt.float32, tag="ps", name="ps_warm"
            )
            NWARM = 32
            for i in range(NWARM):
                nc.tensor.matmul(
                    ps_w[:],
                    lhsT=warm_sb[:, :128],
                    rhs=warm_sb[:],
                    start=(i == 0),
                    stop=(i == NWARM - 1),
                )

            # First W1 pair at the very head of the sync ring so matmuls
            # can start as early as possible.
            w1_tiles = {}
            w1_tiles[0] = w1pool.tile(
                [128, DK, 256], dt.bfloat16, tag="w1", name="w1_t0"
            )
            nc.sync.dma_start(w1_tiles[0][:], W1[:, 0 : DK * 256])

            # per-chunk xT tiles: both DMA sides fully contiguous (8KB runs)
            xT_tiles = []
            for i, (c0, cn) in enumerate(chunks):
                xc = xpool.tile(
                    [128, DK, cn], dt.bfloat16, tag=f"xc{i}", name=f"xc{i}"
                )
                nc.sync.dma_start(
                    xc[:],
                    xT[:, DK * c0 : DK * (c0 + cn)].rearrange(
                        "p (dk c) -> p dk c", dk=DK
                    ),
                )
                xT_tiles.append(xc)
            b1_sb = cpool.tile([128, PK], dt.float32)
            nc.sync.dma_start(b1_sb[:], b1[:])
            wt_sb = cpool.tile([128, TT], dt.float32)
            nc.sync.dma_start(wt_sb[:], wt[:])

            # ---- Phase 1: H = gelu(W1.T x + b1), H layout [p_dim, tokens]
            for pkg in range(PKG):
                if pkg not in w1_tiles:
                    w1_tiles[pkg] = w1pool.tile(
                        [128, DK, 256], dt.bfloat16, tag="w1", name=f"w1_t{pkg}"
                    )
                    nc.sync.dma_start(
                        w1_tiles[pkg][:],
                        W1[:, pkg * DK * 256 : (pkg + 1) * DK * 256],
                    )
                w1_sb = w1_tiles[pkg]
                first_act = None
                for j in range(2):
                    pk = 2 * pkg + j
                    for ci, (c0, cn) in enumerate(chunks):
                        ps = psum_pool.tile([128, 512], dt.float32)
                        for dk in range(DK):
                            nc.tensor.matmul(
                                ps[:, :cn],
                                lhsT=w1_sb[:, dk, j * 128 : (j + 1) * 128],
                                rhs=xT_tiles[ci][:, dk, :],
                                start=(dk == 0),
                                stop=(dk == DK - 1),
                            )
                        act = nc.scalar.activation(
                            H_sb[:, pk, c0 : c0 + cn],
                            ps[:, :cn],
                            act_fn,
                            bias=b1_sb[:, pk : pk + 1],
                        )
                        if first_act is None:
                            first_act = act
                # W2: two pk-chunks per pkg iteration on the scalar ring,
                # gated on this pkg's first activation so the critical
                # early HBM bandwidth all goes to W1/xT.
                w2_dma = nc.scalar.dma_start(
                    W2_sb[:, 2 * pkg : 2 * pkg + 2, :],
                    W2[:, 2 * pkg * D : (2 * pkg + 2) * D].rearrange(
                        "p (k d) -> p k d", k=2
                    ),
                )
                add_dep_helper(
                    w2_dma.ins, first_act.ins, reason="pace W2 behind phase-1"
                )

            # ---- Phase 2: y = (H.T W2) * wt, tokens on partitions
            # (b2 is folded into the host-side combine)
            for tt in range(TT):
                y_sb = ypool.tile([128, D], dt.float32)
                for dn in range(ND):
                    ps = psum_pool.tile([128, 512], dt.float32)
                    for pk in range(PK):
                        nc.tensor.matmul(
                            ps[:],
                            lhsT=H_sb[:, pk, tt * 128 : (tt + 1) * 128],
                            rhs=W2_sb[:, pk, dn * 512 : (dn + 1) * 512],
                            start=(pk == 0),
                            stop=(pk == PK - 1),
                        )
                    nc.scalar.activation(
                        y_sb[:, dn * 512 : (dn + 1) * 512],
                        ps[:],
                        AF.Copy,
                        scale=wt_sb[:, tt : tt + 1],
                    )
                    nc.sync.dma_start(
                        y[tt * 128 : (tt + 1) * 128, dn * 512 : (dn + 1) * 512],
                        y_sb[:, dn * 512 : (dn + 1) * 512],
                    )

    nc.finalize()
    return nc


def _get_nc(C):
    if C not in _NC_CACHE:
        _NC_CACHE[C] = _build_nc(C)
    return _NC_CACHE[C]


def _pack_inputs(xf, te, we, W1e, b1e, W2e, C):
    """Host-side swizzle of one expert's shard into device DRAM layouts."""
    n_e = len(te)
    chunks = _token_chunks(C)

    # xT: [128, DK*C], chunk-major [dk, c] blocks
    xg = np.zeros((C, D), dtype=np.float32)
    xg[:n_e] = xf[te]
    xt = xg.T.astype(BF16).reshape(DK, 128, C)        # [dk, p, c]
    xT_d = np.empty((128, DK * C), dtype=BF16)
    col = 0
    for c0, cn in chunks:
        blk = xt[:, :, c0 : c0 + cn]                  # [dk, p, cn]
        xT_d[:, col : col + DK * cn] = (
            blk.transpose(1, 0, 2).reshape(128, DK * cn)
        )
        col += DK * cn

    # W1: [128, DK*P] as pkg-major blocks [dk, 256]
    w1 = W1e.astype(BF16).reshape(DK, 128, PK // 2, 256)  # [dk, p, pkg, m]
    W1_d = np.ascontiguousarray(
        w1.transpose(1, 2, 0, 3).reshape(128, (PK // 2) * DK * 256)
    )

    # W2: [128, PK*D]
    w2 = W2e.astype(BF16).reshape(PK, 128, D)             # [pk, p, d]
    W2_d = np.ascontiguousarray(w2.transpose(1, 0, 2).reshape(128, PK * D))

    b1_d = np.ascontiguousarray(b1e.astype(np.float32).reshape(PK, 128).T)

    wt_full = np.zeros((C,), dtype=np.float32)
    wt_full[:n_e] = we
    wt_d = np.ascontiguousarray(wt_full.reshape(C // 128, 128).T)

    return {
        "xT": xT_d,
        "W1": W1_d,
        "b1": b1_d,
        "W2": W2_d,
        "wt": wt_d,
    }


def kernel(x, W1, b1, W2, b2, Wr, br):
    from concourse.bass_utils import run_bass_kernel_spmd

    x = np.asarray(x)
    B, S, _ = x.shape
    T = B * S
    xf = np.ascontiguousarray(x.reshape(T, D).astype(np.float32))

    idx, w = _route(xf, np.asarray(Wr, np.float32), np.asarray(br, np.float32))

    # Per-expert token lists
    sel = []
    for e in range(E):
        mask = (idx == e).any(axis=1)
        te = np.nonzero(mask)[0]
        ke = (idx[te] == e).argmax(axis=1)
        we = w[te, ke]
        sel.append((te, we))

    maxn = max(len(te) for te, _ in sel)
    C = ((maxn + 127) // 128) * 128

    nc = _get_nc(C)

    W1f = np.asarray(W1)
    W2f = np.asarray(W2)
    b1f = np.asarray(b1, np.float32)
    b2f = np.asarray(b2, np.float32)

    in_maps = []
    for e in range(E):
        te, we = sel[e]
        in_maps.append(_pack_inputs(xf, te, we, W1f[e], b1f[e], W2f[e], C))

    res = run_bass_kernel_spmd(nc, in_maps, core_ids=list(range(NCORES)))
    global LAST_RESULT
    LAST_RESULT = res

    # Combine: sum per-expert wt*(H@W2) shards, then add the router-weighted
    # b2 term (sum_e w[t,e]*b2[e]) in one tiny [T,E]@[E,D] matmul.
    out = np.zeros((T, D), dtype=np.float32)
    for e in range(E):
        te, _ = sel[e]
        out[te] += res.results[e]["y"][: len(te)]
    w_full = np.zeros((T, E), dtype=np.float32)
    np.put_along_axis(w_full, idx, w, axis=1)
    out += w_full @ b2f
    return out.reshape(B, S, D)


# revision 24
# speedup vs baseline: 1.0334x; 1.0032x over previous
"""MoE block (E=8, top-2, D=1024, P=4096, T=4096) on 8 TRN2 NeuronCores.

Strategy: expert-parallel. The router (0.03% of FLOPs) runs on host to
produce the token->expert dispatch; core e receives the tokens routed to
expert e (gathered, transposed, bf16), runs the expert MLP
  y = (gelu_tanh(x @ W1 + b1) @ W2 + b2) * router_weight
entirely on device, and the host scatter-adds the per-expert outputs back
into the full [T, D] output.

Device kernel (per core, SPMD):
  phase 1: H[p, t] = gelu(sum_d W1[d, p] xT[d, t] + b1[p])   (H kept in SBUF, bf16)
  phase 2: y[t, d] = (sum_p H[p, t] W2[p, d] + b2[d]) * wt[t]
b2 is added via a K=1 ones-row matmul into the same PSUM accumulation
group; the router weight is applied as a per-partition activation scale.

All DRAM inputs are pre-swizzled on host so every device DMA reads fully
contiguous per-partition runs (the partition index p is the SLOWEST axis,
matching SBUF tile layout):
  xT_d [128, DK*C]     xT_d[p, chunk-major (dk, c)] = x_g[c, dk*128+p]
  W1_d [128, DK*P]     blocks of [DK, 2*128] per pk-pair (pkg-major)
  W2_d [128, PK*D]     W2_d[p, pk*D + d] = W2[pk*128+p, d]
  b1_d [128, PK]       b1_d[p, pk] = b1[pk*128+p]
  wt_d [128, TT]       wt_d[p, tt] = w[tt*128+p]
"""

import numpy as np
import ml_dtypes

E = 8
K = 2
D = 1024
P = 4096
NCORES = 8

DK = D // 128   # 8
PK = P // 128   # 32

BF16 = ml_dtypes.bfloat16

_NC_CACHE = {}


def _route(xf, Wr, br):
    """Top-2 routing + softmax weights, matching the jax reference."""
    scores = xf @ Wr + br                                   # [T, E] fp32
    idx = np.argsort(-scores, axis=-1, kind="stable")[:, :K]  # [T, K]
    top = np.take_along_axis(scores, idx, axis=-1)          # [T, K]
    m = top.max(axis=-1, keepdims=True)
    ex = np.exp(top - m)
    w = ex / ex.sum(axis=-1, keepdims=True)                 # [T, K]
    return idx, w


def _token_chunks(C):
    """Split C into free-dim chunks of <=512 for fp32 PSUM banks.

    The first chunk is only 128 tokens so the very first matmul group
    depends on a minimal amount of DMA'd data.
    """
    chunks = [(0, 128)]
    c0 = 128
    while c0 < C:
        cn = min(512, C - c0)
        chunks.append((c0, cn))
        c0 += cn
    return chunks


def _build_nc(C, act_fn=None):
    """Build the per-core Bass graph for capacity-C tokens."""
    import concourse.bass as bass  # noqa: F401
    import concourse.mybir as mybir
    import concourse.tile as tile
    from concourse.tile import add_dep_helper
    from concourse import bacc

    dt = mybir.dt
    AF = mybir.ActivationFunctionType
    if act_fn is None:
        act_fn = AF.Gelu_apprx_tanh

    TT = C // 128    # token tiles in phase 2
    ND = D // 512    # 2 output d-chunks in phase 2
    PKG = PK // 2    # W1 streamed in pk-pairs for 4KB-contiguous DMA

    nc = bacc.Bacc(None, target_bir_lowering=False)

    xT = nc.dram_tensor("xT", [128, DK * C], dt.bfloat16, kind="ExternalInput")
    W1 = nc.dram_tensor("W1", [128, DK * P], dt.bfloat16, kind="ExternalInput")
    b1 = nc.dram_tensor("b1", [128, PK], dt.float32, kind="ExternalInput")
    W2 = nc.dram_tensor("W2", [128, PK * D], dt.bfloat16, kind="ExternalInput")
    wt = nc.dram_tensor("wt", [128, TT], dt.float32, kind="ExternalInput")
    y = nc.dram_tensor("y", [C, D], dt.float32, kind="ExternalOutput")

    chunks = _token_chunks(C)

    with tile.TileContext(nc) as tc:
        with (
            tc.tile_pool(name="xpool", bufs=1) as xpool,
            tc.tile_pool(name="w1pool", bufs=4) as w1pool,
            tc.tile_pool(name="w2pool", bufs=1) as w2pool,
            tc.tile_pool(name="hpool", bufs=1) as hpool,
            tc.tile_pool(name="cpool", bufs=1) as cpool,
            tc.tile_pool(name="ypool", bufs=3) as ypool,
            tc.tile_pool(name="psum", bufs=8, space="PSUM") as psum_pool,
        ):
            H_sb = hpool.tile([128, PK, C], dt.bfloat16)
            W2_sb = w2pool.tile([128, PK, D], dt.bfloat16)

            # PE warm-up: ~5us of dummy matmuls with no DMA dependency so
            # the HAM clock-gate opens (1.2 -> 2.4 GHz) while the first
            # real tiles are still in flight on the DMA rings.
            warm_sb = cpool.tile([128, 512], dt.bfloat16)
            nc.any.memset(warm_sb[:], 0.0)
            ps_w = psum_pool.tile(
                [128, 512], dt.float32, tag="ps", name="ps_warm"
            )
            NWARM = 30
            for i in range(NWARM):
                nc.tensor.matmul(
                    ps_w[:, :256],
                    lhsT=warm_sb[:, :128],
                    rhs=warm_sb[:, :256],
                    start=(i == 0),
                    stop=(i == NWARM - 1),
                )

            # First W1 pair at the very head of the sync ring so matmuls
            # can start as early as possible.
            w1_tiles = {}
            w1_tiles[0] = w1pool.tile(
                [128, DK, 256], dt.bfloat16, tag="w1", name="w1_t0"
            )
            nc.sync.dma_start(w1_tiles[0][:], W1[:, 0 : DK * 256])

            # per-chunk xT tiles: both DMA sides fully contiguous (8KB runs)
            xT_tiles = []
            for i, (c0, cn) in enumerate(chunks):
                xc = xpool.tile(
                    [128, DK, cn], dt.bfloat16, tag=f"xc{i}", name=f"xc{i}"
                )
                nc.sync.dma_start(
                    xc[:],
                    xT[:, DK * c0 : DK * (c0 + cn)].rearrange(
                        "p (dk c) -> p dk c", dk=DK
                    ),
                )
                xT_tiles.append(xc)
            b1_sb = cpool.tile([128, PK], dt.float32)
            nc.sync.dma_start(b1_sb[:], b1[:])
            wt_sb = cpool.tile([128, TT], dt.float32)
            nc.sync.dma_start(wt_sb[:], wt[:])

            # ---- Phase 1: H = gelu(W1.T x + b1), H layout [p_dim, tokens]
            for pkg in range(PKG):
                if pkg not in w1_tiles:
                    w1_tiles[pkg] = w1pool.tile(
                        [128, DK, 256], dt.bfloat16, tag="w1", name=f"w1_t{pkg}"
                    )
                    nc.sync.dma_start(
                        w1_tiles[pkg][:],
                        W1[:, pkg * DK * 256 : (pkg + 1) * DK * 256],
                    )
                w1_sb = w1_tiles[pkg]
                first_act = None
                for j in range(2):
                    pk = 2 * pkg + j
                    for ci, (c0, cn) in enumerate(chunks):
                        ps = psum_pool.tile([128, 512], dt.float32)
                        for dk in range(DK):
                            nc.tensor.matmul(
                                ps[:, :cn],
                                lhsT=w1_sb[:, dk, j * 128 : (j + 1) * 128],
                                rhs=xT_tiles[ci][:, dk, :],
                                start=(dk == 0),
                                stop=(dk == DK - 1),
                            )
                        act = nc.scalar.activation(
                            H_sb[:, pk, c0 : c0 + cn],
                            ps[:, :cn],
                            act_fn,
                            bias=b1_sb[:, pk : pk + 1],
                        )
                        if first_act is None:
                            first_act = act
                # W2: two pk-chunks per pkg iteration on the scalar ring,
                # gated on this pkg's first activation so the critical
                # early HBM bandwidth all goes to W1/xT.
                w2_dma = nc.scalar.dma_start(
                    W2_sb[:, 2 * pkg : 2 * pkg + 2, :],
                    W2[:, 2 * pkg * D : (2 * pkg + 2) * D].rearrange(
                        "p (k d) -> p k d", k=2
                    ),
                )
                add_dep_helper(
                    w2_dma.ins, first_act.ins, reason="pace W2 behind phase-1"
                )

            # ---- Phase 2: y = (H.T W2) * wt, tokens on partitions
            # (b2 is folded into the host-side combine)
            for tt in range(TT):
                y_sb = ypool.tile([128, D], dt.float32)
                for dn in range(ND):
                    ps = psum_pool.tile([128, 512], dt.float32)
                    for pk in range(PK):
                        nc.tensor.matmul(
                            ps[:],
                            lhsT=H_sb[:, pk, tt * 128 : (tt + 1) * 128],
                            rhs=W2_sb[:, pk, dn * 512 : (dn + 1) * 512],
                            start=(pk == 0),
                            stop=(pk == PK - 1),
                        )
                    nc.scalar.activation(
                        y_sb[:, dn * 512 : (dn + 1) * 512],
                        ps[:],
                        AF.Copy,
                        scale=wt_sb[:, tt : tt + 1],
                    )
                    nc.sync.dma_start(
                        y[tt * 128 : (tt + 1) * 128, dn * 512 : (dn + 1) * 512],
                        y_sb[:, dn * 512 : (dn + 1) * 512],
                    )

    nc.finalize()
    return nc


def _get_nc(C):
    if C not in _NC_CACHE:
        _NC_CACHE[C] = _build_nc(C)
    return _NC_CACHE[C]


def _pack_inputs(xf, te, we, W1e, b1e, W2e, C):
    """Host-side swizzle of one expert's shard into device DRAM layouts."""
    n_e = len(te)
    chunks = _token_chunks(C)

    # xT: [128, DK*C], chunk-major [dk, c] blocks
    xg = np.zeros((C, D), dtype=np.float32)
    xg[:n_e] = xf[te]
    xt = xg.T.astype(BF16).reshape(DK, 128, C)        # [dk, p, c]
    xT_d = np.empty((128, DK * C), dtype=BF16)
    col = 0
    for c0, cn in chunks:
        blk = xt[:, :, c0 : c0 + cn]                  # [dk, p, cn]
        xT_d[:, col : col + DK * cn] = (
            blk.transpose(1, 0, 2).reshape(128, DK * cn)
        )
        col += DK * cn

    # W1: [128, DK*P] as pkg-major blocks [dk, 256]
    w1 = W1e.astype(BF16).reshape(DK, 128, PK // 2, 256)  # [dk, p, pkg, m]
    W1_d = np.ascontiguousarray(
        w1.transpose(1, 2, 0, 3).reshape(128, (PK // 2) * DK * 256)
    )

    # W2: [128, PK*D]
    w2 = W2e.astype(BF16).reshape(PK, 128, D)             # [pk, p, d]
    W2_d = np.ascontiguousarray(w2.transpose(1, 0, 2).reshape(128, PK * D))

    b1_d = np.ascontiguousarray(b1e.astype(np.float32).reshape(PK, 128).T)

    wt_full = np.zeros((C,), dtype=np.float32)
    wt_full[:n_e] = we
    wt_d = np.ascontiguousarray(wt_full.reshape(C // 128, 128).T)

    return {
        "xT": xT_d,
        "W1": W1_d,
        "b1": b1_d,
        "W2": W2_d,
        "wt": wt_d,
    }


def kernel(x, W1, b1, W2, b2, Wr, br):
    from concourse.bass_utils import run_bass_kernel_spmd

    x = np.asarray(x)
    B, S, _ = x.shape
    T = B * S
    xf = np.ascontiguousarray(x.reshape(T, D).astype(np.float32))

    idx, w = _route(xf, np.asarray(Wr, np.float32), np.asarray(br, np.float32))

    # Per-expert token lists
    sel = []
    for e in range(E):
        mask = (idx == e).any(axis=1)
        te = np.nonzero(mask)[0]
        ke = (idx[te] == e).argmax(axis=1)
        we = w[te, ke]
        sel.append((te, we))

    maxn = max(len(te) for te, _ in sel)
    C = ((maxn + 127) // 128) * 128

    nc = _get_nc(C)

    W1f = np.asarray(W1)
    W2f = np.asarray(W2)
    b1f = np.asarray(b1, np.float32)
    b2f = np.asarray(b2, np.float32)

    in_maps = []
    for e in range(E):
        te, we = sel[e]
        in_maps.append(_pack_inputs(xf, te, we, W1f[e], b1f[e], W2f[e], C))

    res = run_bass_kernel_spmd(nc, in_maps, core_ids=list(range(NCORES)))
    global LAST_RESULT
    LAST_RESULT = res

    # Combine: sum per-expert wt*(H@W2) shards, then add the router-weighted
    # b2 term (sum_e w[t,e]*b2[e]) in one tiny [T,E]@[E,D] matmul.
    out = np.zeros((T, D), dtype=np.float32)
    for e in range(E):
        te, _ = sel[e]
        out[te] += res.results[e]["y"][: len(te)]
    w_full = np.zeros((T, E), dtype=np.float32)
    np.put_along_axis(w_full, idx, w, axis=1)
    out += w_full @ b2f
    return out.reshape(B, S, D)
